# revision 20
# baseline (speedup 1.0000x reference)
"""GAT self-attention kernel for Trainium2 (8 NeuronCores, SPMD data-parallel over N).

Math (per graph n):
  h_t = X @ W_t ; q_gate_t = sigmoid(relu(q @ W1_t) @ W2_t)
  s_src_t = (h_t * g1) @ a1 ; s_dst_t = (h_t * g2) @ a2
  score[i,j] = lrelu(s_src_{adj[i,j]-1}[i] + s_dst_{adj[i,j]-1}[j])   (adj>0)
  out = softmax_j(score) @ (h_3 * node_mask)

Device strategy (v3):
  - The tiny q-gate MLP and the per-type fold (gate*a through W_t^T, then
    through the Vinv cubic-interpolation basis) run on the HOST; the device
    receives W3CP = [W_3 | per-graph src/dst cubic-coefficient columns] and
    computes h_3 plus all per-row polynomial coefficients in one fused matmul.
  - The 4-way type select over adj is a cubic in z = adj - 2.5 evaluated per
    cell: src side in [i-part, j-free] (ACT pre-step + custom DVE Horner,
    bf16), PE-transposed (bf16, cheap) into PSUM; dst side in [j-part, i-free]
    (gpsimd pre-step + custom DVE Horner with the adj>0 mask emitting -3e4).
  - combine is a builtin DVE tensor-tensor add (PSUM-bf16 + SBUF-bf16, 2x
    mode), then Prelu and Exp on the ACT engine (one table set, no reloads).
  - node_mask folds into hm = h_3 * m during the PSUM->SBUF copy (ACT Copy
    with per-partition scale); a ones column in hm recovers the softmax
    denominator through the final matmul; normalization happens on the host
    (the kernel DMAs numerator+denominator straight from PSUM as f32).
"""

import numpy as np
from contextlib import ExitStack

import concourse.bass as bass
import concourse.bacc as bacc
import concourse.tile as tile
from concourse import mybir
from concourse import dve_ops
from concourse.dve_spec import (Spec, Src0, Src1, C0, C1, C2, MaxNeg, Zero,
                                One, select)
from concourse.dve_uop import DveOpSpec
from concourse.bass_utils import run_bass_kernel_spmd


def _register_dve_op(name, spec):
    """Runtime-register a custom DVE op (fp32-internal fused pipeline)."""
    if name in dve_ops._SUB_OPCODE_FOR_NAME:
        return dve_ops.CUSTOM_DVE_SPECS[name + "_OP"]
    op = dve_ops.DveOp(name, spec, subdim=False, uops_sha={},
                       perf_en={"v3": True, "v4": True})
    dve_ops.OPS.append(op)
    dve_ops.CUSTOM_DVE_SPECS[name] = spec
    dve_ops._SUB_OPCODE_FOR_NAME[name] = (
        max(dve_ops._SUB_OPCODE_FOR_NAME.values()) + 1)
    shas = {}
    for ver in ("v3", "v4"):
        s = DveOpSpec(
            name=name,
            opcode=dve_ops.get_dve_sub_opcode(name),
            uops=dve_ops.lower(spec, ver=ver),
            rd1_en=dve_ops.has_src1(spec),
        ).sha(ver)
        shas[ver] = s
    object.__setattr__(op, "uops_sha", shas)
    dve_ops.CUSTOM_DVE_SPECS[name + "_OP"] = op
    return op


def _register_horner():
    # out = (in0*in1 + s0)*in1 + s1 : cubic tail given t1 = a3*z + a2
    return _register_dve_op("HORNER2A_ANT", Spec(
        body=(Src0 * Src1 + C0) * Src1 + C1,
        reference=lambda in0, in1, s0, s1, imm2: (in0 * in1 + s0) * in1 + s1,
    ))


def _register_hornerm():
    # masked cubic tail: imm2 (a large negative, bf16-safe) where
    # in1 (= adj-2.5) <= -2, i.e. adj == 0
    return _register_dve_op("HORNERM_ANT", Spec(
        body=select(Src1 > (Zero - (One + One)),
                    (Src0 * Src1 + C0) * Src1 + C1, C2),
        reference=lambda in0, in1, s0, s1, imm2: np.where(
            in1 > -2.0, (in0 * in1 + s0) * in1 + s1, imm2),
    ))


f32 = mybir.dt.float32
f32r = mybir.dt.float32r
bf16 = mybir.dt.bfloat16
fp8 = mybir.dt.float8e4
Alu = mybir.AluOpType
Act = mybir.ActivationFunctionType

N, E, D, NT = 32, 512, 300, 4
NCORES = 8
GPC = N // NCORES  # graphs per core
SLOPE = 0.2
MASKVAL = -30000.0

DC3 = [(0, 128), (128, 128), (256, 44)]           # 300 split into <=128 chunks
EC4 = [(i * 128, 128) for i in range(4)]          # 512 split into 4 chunks
DW = D + 32                                        # 332 W3CP columns
DH = D + 1                                         # 301 hm columns (ones col)

# engine assignment for the dst pre-step (per jj chunk): True -> gpsimd
DST_PRE_GP = [False, True, True, True]


def build_nc():
    nc = bacc.Bacc("TRN2", target_bir_lowering=False, debug=False,
                   enable_partition_id=True)

    def din(name, shape, dt=f32):
        return nc.dram_tensor(name, shape, dt, kind="ExternalInput").ap()

    identB = din("identB", [128, 128], bf16)
    w3cp = din("w3cp", [128, 3, DW], bf16)
    mcol = din("mcol", [128, GPC * 4])
    xT = din("xT", [GPC, 3, 128, E], bf16)        # input_state[n].T, padded
    adjAB = din("adjAB", [GPC, 2, E, E], fp8)     # [adj-2.5, adj.T-2.5]
    out = nc.dram_tensor("out", [GPC, 4, 128, D], bf16,
                         kind="ExternalOutput").ap()

    with tile.TileContext(nc) as tc:
        with ExitStack() as ctx:
            _body(ctx, tc, identB, w3cp, mcol, xT, adjAB, out)
    nc.compile()
    return nc


def _body(ctx, tc, identB, w3cp, mcol, xT, adjAB, out):
    nc = tc.nc
    HORNER = _register_horner()
    HORNERM = _register_hornerm()
    const = ctx.enter_context(tc.tile_pool(name="const", bufs=1))
    xpool = ctx.enter_context(tc.tile_pool(name="xpool", bufs=1))
    adjp = ctx.enter_context(tc.tile_pool(name="adjp", bufs=3))
    adjtp = ctx.enter_context(tc.tile_pool(name="adjtp", bufs=2))
    t1p = ctx.enter_context(tc.tile_pool(name="t1p", bufs=6))
    pip = ctx.enter_context(tc.tile_pool(name="pip", bufs=12))
    pbmp = ctx.enter_context(tc.tile_pool(name="pbmp", bufs=4))
    scp = ctx.enter_context(tc.tile_pool(name="scp", bufs=3))
    lrp = ctx.enter_context(tc.tile_pool(name="lrp", bufs=3))
    ehp = ctx.enter_context(tc.tile_pool(name="ehp", bufs=3))
    outp = ctx.enter_context(tc.tile_pool(name="outp", bufs=2))
    otp = ctx.enter_context(tc.tile_pool(name="otp", bufs=3))
    hmp = ctx.enter_context(tc.tile_pool(name="hmp", bufs=1))
    ckp = ctx.enter_context(tc.tile_pool(name="ckp", bufs=1))
    psq = ctx.enter_context(tc.tile_pool(name="psq", bufs=2, space="PSUM"))
    ztp = ctx.enter_context(tc.tile_pool(name="ztp", bufs=3, space="PSUM"))
    pop = ctx.enter_context(tc.tile_pool(name="pop", bufs=3, space="PSUM"))

    # ---- constant + weight loads (sync queue, in consumption order) ----
    IDB = const.tile([128, 128], bf16)
    nc.sync.dma_start(out=IDB, in_=identB)
    W3CP = const.tile([128, 3, DW], bf16)
    nc.sync.dma_start(out=W3CP, in_=w3cp)
    XTs = []
    for n in range(GPC):
        XTn = xpool.tile([128, 3, E], bf16, tag=f"xt_{n}")
        XTs.append(XTn)
    nc.sync.dma_start(out=XTs[0][:, :, 0:128],
                      in_=xT[0][:, :, 0:128].rearrange("c p e -> p c e"))
    nc.sync.dma_start(out=XTs[0][:, :, 128:E],
                      in_=xT[0][:, :, 128:E].rearrange("c p e -> p c e"))
    MC = const.tile([128, GPC * 4], f32)
    nc.sync.dma_start(out=MC, in_=mcol)
    # PE p-state warmup: dummy matmuls on the identity while inputs stream
    # (one static tile so the psq ring stays free for phase-1)
    pw = ztp.tile([128, E], f32, tag="zt", name="PW")
    for w in range(8):
        nc.tensor.matmul(pw[:, 0:128], IDB, IDB, start=True, stop=True,
                         skip_group_check=True)
    ABs = {}

    def load_adj(n):
        AB = adjp.tile([128, 2, 4, E], fp8, tag="ab", name="AB")
        nc.sync.dma_start(out=AB,
                          in_=adjAB[n].rearrange("b (c p) e -> p b c e", p=128))
        ABs[n] = AB

    def load_x(n):
        nc.sync.dma_start(out=XTs[n], in_=xT[n].rearrange("c p e -> p c e"))

    load_adj(0)
    load_x(1)
    load_adj(1)

    # hm tiles: ones column written once per buffer (16 tiles <-> 16 bufs)
    HMs, CKs = {}, {}
    for n in range(GPC):
        HMs[n] = []
        for ii in range(4):
            hm = hmp.tile([128, DH], bf16, tag=f"hm_{n}_{ii}")
            nc.gpsimd.memset(hm[:, D:DH], 1.0)
            HMs[n].append(hm)

    # ---- phase A: h3 + all poly coefficients in one matmul per e-chunk ----
    def emit_A(n):
        CK = []
        for ii, (eo, el) in enumerate(EC4):
            psh = psq.tile([128, DW], f32, tag="ph")
            for ci, (do, dl) in enumerate(DC3):
                nc.tensor.matmul(psh, XTs[n][:dl, ci, eo:eo + el],
                                 W3CP[:dl, ci, :],
                                 start=(ci == 0), stop=(ci == 2))
            hm = HMs[n][ii]
            nc.scalar.mul(hm[:, 0:D], psh[:, 0:D],
                          MC[:, n * 4 + ii:n * 4 + ii + 1])
            ck = ckp.tile([128, 8], f32, tag=f"ck_{n}_{ii}")
            nc.vector.tensor_copy(ck, psh[:, D + 8 * n:D + 8 * n + 8])
            CK.append(ck)
        CKs[n] = CK

    # ---- phase C: src cubic in [i-part, j-free] ----
    PSIs = {}

    def emit_C(n):
        AJ4 = ABs[n]
        PS_I = []
        for ii in range(4):
            aj = AJ4[:, 0, ii, :]
            ck = CKs[n][ii]
            t1 = t1p.tile([128, E], bf16, tag="t1")
            nc.gpsimd.tensor_scalar(t1, aj, ck[:, 3:4], ck[:, 2:3],
                                    Alu.mult, Alu.add)
            pi = pip.tile([128, E], bf16, tag="pi")
            nc.vector._custom_dve(HORNER, out=pi, in0=t1, in1=aj,
                                  s0=ck[:, 1:2], s1=ck[:, 0:1])
            PS_I.append(pi)
        PSIs[n] = PS_I

    # ---- phase T: PE transposes (bf16) into PSUM bf16 views ----
    ZTs = {}

    def emit_T(n):
        PS_I = PSIs[n]
        zv = []
        for half in range(2):
            ZT = ztp.tile([128, E], f32, tag="zt", name="ZT")
            ztv = ZT.bitcast(bf16)  # [128, 1024]
            for sub in range(2):
                jj = half * 2 + sub
                for ii, (eo, el) in enumerate(EC4):
                    nc.tensor.matmul(
                        ztv[:, sub * E + ii * 128: sub * E + ii * 128 + 128],
                        PS_I[ii][:, jj * 128:jj * 128 + 128], IDB,
                        is_transpose=True, start=True, stop=True,
                        skip_group_check=True)
            zv.append(ztv)
        ZTs[n] = zv

    # ---- phase D: dst cubic + combine + lrelu + exp in [j-part, i-free] ----
    EHs, SCs, LRs = {}, {}, {}

    def emit_D(n, half):
        AT4, CK = ABs[n], CKs[n]
        zv = ZTs[n]
        if half == 0:
            SCs[n] = scp.tile([128, 4, E], bf16, tag="sc", name="SC")
            LRs[n] = lrp.tile([128, 4, E], bf16, tag="lr", name="LR")
            EHs[n] = ehp.tile([128, 4, E], bf16, tag="eh", name="EH")
        SC, LR, EH = SCs[n], LRs[n], EHs[n]
        for jj in (2 * half, 2 * half + 1):
            at = AT4[:, 1, jj, :]
            ck = CK[jj]
            t1b = t1p.tile([128, E], bf16, tag="t1b")
            if DST_PRE_GP[jj] and n < 3:
                nc.gpsimd.tensor_scalar(t1b, at, ck[:, 7:8], ck[:, 6:7],
                                        Alu.mult, Alu.add)
            else:
                nc.vector.tensor_scalar(t1b, at, ck[:, 7:8], ck[:, 6:7],
                                        Alu.mult, Alu.add)
            pbm = pbmp.tile([128, E], bf16, tag="pbm")
            nc.vector._custom_dve(HORNERM, out=pbm, in0=t1b, in1=at,
                                  s0=ck[:, 5:6], s1=ck[:, 4:5], imm2=MASKVAL)
            ztv = zv[jj // 2]
            nc.vector.tensor_tensor(
                SC[:, jj, :],
                ztv[:, (jj % 2) * E:(jj % 2) * E + E].bitcast(bf16),
                pbm, Alu.add)
        h2 = 2 * half
        nc.scalar.activation(LR[:, h2:h2 + 2, :], SC[:, h2:h2 + 2, :],
                             Act.Prelu, alpha=SLOPE)
        nc.scalar.activation(EH[:, h2:h2 + 2, :], LR[:, h2:h2 + 2, :],
                             Act.Exp)

    # ---- phase E: final matmul + normalize (in ii-halves) ----
    OTs = {}

    def emit_E(n, half):
        EH, HM = EHs[n], HMs[n]
        if half == 0:
            OTs[n] = otp.tile([128, 4, D], bf16, tag="ot", name="OT")
        OT = OTs[n]
        i0, i1 = 2 * half, 2 * half + 1
        poA = pop.tile([128, DH], f32, tag="po", name="poA")
        poB = pop.tile([128, DH], f32, tag="po", name="poB")
        for jj in range(4):
            nc.tensor.matmul(poA, EH[:, jj, EC4[i0][0]:EC4[i0][0] + 128],
                             HM[jj][:, 0:DH],
                             start=(jj == 0), stop=(jj == 3),
                             skip_group_check=True)
            nc.tensor.matmul(poB, EH[:, jj, EC4[i1][0]:EC4[i1][0] + 128],
                             HM[jj][:, 0:DH],
                             start=(jj == 0), stop=(jj == 3),
                             skip_group_check=True)
        for ii, po in ((i0, poA), (i1, poB)):
            rc = outp.tile([128, 1], f32, tag="rc")
            nc.vector.reciprocal(rc, po[:, D:D + 1])
            nc.scalar.mul(OT[:, ii, :], po[:, 0:D], rc)
        eng = nc.scalar if n == 3 else nc.sync
        eng.dma_start(
            out=out[n, 2 * half:2 * half + 2].rearrange("c p d -> p c d"),
            in_=OT[:, 2 * half:2 * half + 2, :])

    # software-pipelined emission: 2 graphs in flight, half-graph granularity
    emit_A(0)
    emit_C(0)
    emit_A(1)
    emit_T(0)
    emit_D(0, 0)
    load_x(2)
    emit_C(1)
    emit_D(0, 1)
    load_adj(2)
    emit_T(1)
    emit_E(0, 0)
    emit_D(1, 0)
    emit_E(0, 1)
    emit_D(1, 1)
    emit_A(2)
    load_x(3)
    emit_C(2)
    load_adj(3)
    emit_T(2)
    emit_E(1, 0)
    emit_D(2, 0)
    emit_E(1, 1)
    emit_A(3)
    emit_D(2, 1)
    emit_C(3)
    emit_T(3)
    emit_D(3, 0)
    emit_E(2, 0)
    emit_D(3, 1)
    emit_E(2, 1)
    emit_E(3, 0)
    emit_E(3, 1)


def _vinv():
    # centered basis z = adj - 2.5: coeffs a0..a3 of the cubic through
    # (z_t, u_t), z_t in {-1.5,-0.5,0.5,1.5} (well conditioned, exact bf16)
    V = np.array([[((t + 1) - 2.5) ** m for m in range(4)] for t in range(4)],
                 np.float64)
    return np.linalg.inv(V)


def _prep_inputs(input_state, adj, node_mask, query_vec, W_type, a_type,
                 qattn_W1, qattn_W2):
    import ml_dtypes
    bf = ml_dtypes.bfloat16
    f8 = ml_dtypes.float8_e4m3fn
    X = np.asarray(input_state, np.float32)
    A = np.asarray(adj, np.int32)
    NMsk = np.asarray(node_mask, np.float32)
    Q = np.asarray(query_vec, np.float64)
    W = np.asarray(W_type, np.float64)
    AV = np.asarray(a_type, np.float64)
    W1 = np.asarray(qattn_W1, np.float64)
    W2 = np.asarray(qattn_W2, np.float64)

    # host: q-gate MLP + fold gate*a through W_t^T, then Vinv cubic basis
    Vsrc = np.zeros((N, NT, D))
    Vdst = np.zeros((N, NT, D))
    for t in range(NT):
        r = np.maximum(Q @ W1[t], 0.0)
        g = 1.0 / (1.0 + np.exp(-(r @ W2[t])))      # [N, 600]
        g1, g2 = g[:, :D], g[:, D:]
        a1, a2 = AV[t][:D], AV[t][D:]
        Vsrc[:, t] = (g1 * a1) @ W[t].T
        Vdst[:, t] = (g2 * a2) @ W[t].T
    Vi = _vinv()
    Csrc = np.einsum("kt,ntd->nkd", Vi, Vsrc)       # [N,4,300]
    Cdst = np.einsum("kt,ntd->nkd", Vi, Vdst)

    identB = np.ascontiguousarray(np.eye(128, dtype=np.float32)).astype(bf)
    ZA = (A.astype(np.float32) - 2.5).astype(f8)                  # [N,E,E]
    ZB = np.ascontiguousarray(
        (A.transpose(0, 2, 1).astype(np.float32) - 2.5)).astype(f8)
    ZAB = np.ascontiguousarray(np.stack([ZA, ZB], axis=1))        # [N,2,E,E]
    XP = np.zeros((N, 3, 128, E), np.float32)
    XT_ = X.transpose(0, 2, 1)                                    # [N,300,E]
    for ci, (do, dl) in enumerate(DC3):
        XP[:, ci, 0:dl, :] = XT_[:, do:do + dl, :]

    in_maps = []
    for c in range(NCORES):
        sl = slice(c * GPC, (c + 1) * GPC)
        # W3CP pack: W_3 rows + per-graph coefficient columns
        w3cp = np.zeros((128, 3, DW), np.float32)
        for ci, (do, dl) in enumerate(DC3):
            w3cp[:dl, ci, 0:D] = W[NT - 1, do:do + dl, :]
            for g in range(GPC):
                n = c * GPC + g
                for k in range(4):
                    w3cp[:dl, ci, D + 8 * g + k] = Csrc[n, k, do:do + dl]
                    w3cp[:dl, ci, D + 8 * g + 4 + k] = Cdst[n, k, do:do + dl]
        Mc = np.maximum(NMsk[sl, :, 0], 0.0)        # [GPC, 512]
        mcol = np.zeros((128, GPC * 4), np.float32)
        for g in range(GPC):
            for jj in range(4):
                mcol[:, g * 4 + jj] = Mc[g, jj * 128:(jj + 1) * 128]
        in_maps.append({
            "identB": identB,
            "w3cp": w3cp.astype(bf),
            "mcol": mcol,
            "xT": np.ascontiguousarray(XP[sl]).astype(bf),
            "adjAB": ZAB[sl],
        })
    return in_maps


_NC_CACHE = {}


def kernel(**inputs):
    if "nc" not in _NC_CACHE:
        _NC_CACHE["nc"] = build_nc()
    nc = _NC_CACHE["nc"]
    in_maps = _prep_inputs(**inputs)
    res = run_bass_kernel_spmd(nc, in_maps, list(range(NCORES)))
    outs = []
    for c in range(NCORES):
        o = np.asarray(res.results[c]["out"]).astype(np.float32)
        outs.append(o.reshape(GPC, E, D))
    return np.concatenate(outs, axis=0).astype(np.float32)


# revision 21
# speedup vs baseline: 1.0013x; 1.0013x over previous
"""GAT self-attention kernel for Trainium2 (8 NeuronCores, SPMD data-parallel over N).

Math (per graph n):
  h_t = X @ W_t ; q_gate_t = sigmoid(relu(q @ W1_t) @ W2_t)
  s_src_t = (h_t * g1) @ a1 ; s_dst_t = (h_t * g2) @ a2
  score[i,j] = lrelu(s_src_{adj[i,j]-1}[i] + s_dst_{adj[i,j]-1}[j])   (adj>0)
  out = softmax_j(score) @ (h_3 * node_mask)

Device strategy (v3):
  - The tiny q-gate MLP and the per-type fold (gate*a through W_t^T, then
    through the Vinv cubic-interpolation basis) run on the HOST; the device
    receives W3CP = [W_3 | per-graph src/dst cubic-coefficient columns] and
    computes h_3 plus all per-row polynomial coefficients in one fused matmul.
  - The 4-way type select over adj is a cubic in z = adj - 2.5 evaluated per
    cell: src side in [i-part, j-free] (ACT pre-step + custom DVE Horner,
    bf16), PE-transposed (bf16, cheap) into PSUM; dst side in [j-part, i-free]
    (gpsimd pre-step + custom DVE Horner with the adj>0 mask emitting -3e4).
  - combine is a builtin DVE tensor-tensor add (PSUM-bf16 + SBUF-bf16, 2x
    mode), then Prelu and Exp on the ACT engine (one table set, no reloads).
  - node_mask folds into hm = h_3 * m during the PSUM->SBUF copy (ACT Copy
    with per-partition scale); a ones column in hm recovers the softmax
    denominator through the final matmul; normalization happens on the host
    (the kernel DMAs numerator+denominator straight from PSUM as f32).
"""

import numpy as np
from contextlib import ExitStack

import concourse.bass as bass
import concourse.bacc as bacc
import concourse.tile as tile
from concourse import mybir
from concourse import dve_ops
from concourse.dve_spec import (Spec, Src0, Src1, C0, C1, C2, MaxNeg, Zero,
                                One, select)
from concourse.dve_uop import DveOpSpec
from concourse.bass_utils import run_bass_kernel_spmd


def _register_dve_op(name, spec):
    """Runtime-register a custom DVE op (fp32-internal fused pipeline)."""
    if name in dve_ops._SUB_OPCODE_FOR_NAME:
        return dve_ops.CUSTOM_DVE_SPECS[name + "_OP"]
    op = dve_ops.DveOp(name, spec, subdim=False, uops_sha={},
                       perf_en={"v3": True, "v4": True})
    dve_ops.OPS.append(op)
    dve_ops.CUSTOM_DVE_SPECS[name] = spec
    dve_ops._SUB_OPCODE_FOR_NAME[name] = (
        max(dve_ops._SUB_OPCODE_FOR_NAME.values()) + 1)
    shas = {}
    for ver in ("v3", "v4"):
        s = DveOpSpec(
            name=name,
            opcode=dve_ops.get_dve_sub_opcode(name),
            uops=dve_ops.lower(spec, ver=ver),
            rd1_en=dve_ops.has_src1(spec),
        ).sha(ver)
        shas[ver] = s
    object.__setattr__(op, "uops_sha", shas)
    dve_ops.CUSTOM_DVE_SPECS[name + "_OP"] = op
    return op


def _register_horner():
    # out = (in0*in1 + s0)*in1 + s1 : cubic tail given t1 = a3*z + a2
    return _register_dve_op("HORNER2A_ANT", Spec(
        body=(Src0 * Src1 + C0) * Src1 + C1,
        reference=lambda in0, in1, s0, s1, imm2: (in0 * in1 + s0) * in1 + s1,
    ))


def _register_hornerm():
    # masked cubic tail: imm2 (a large negative, bf16-safe) where
    # in1 (= adj-2.5) <= -2, i.e. adj == 0
    return _register_dve_op("HORNERM_ANT", Spec(
        body=select(Src1 > (Zero - (One + One)),
                    (Src0 * Src1 + C0) * Src1 + C1, C2),
        reference=lambda in0, in1, s0, s1, imm2: np.where(
            in1 > -2.0, (in0 * in1 + s0) * in1 + s1, imm2),
    ))


f32 = mybir.dt.float32
f32r = mybir.dt.float32r
bf16 = mybir.dt.bfloat16
fp8 = mybir.dt.float8e4
Alu = mybir.AluOpType
Act = mybir.ActivationFunctionType

N, E, D, NT = 32, 512, 300, 4
NCORES = 8
GPC = N // NCORES  # graphs per core
SLOPE = 0.2
MASKVAL = -30000.0

DC3 = [(0, 128), (128, 128), (256, 44)]           # 300 split into <=128 chunks
EC4 = [(i * 128, 128) for i in range(4)]          # 512 split into 4 chunks
DW = D + 32                                        # 332 W3CP columns
DH = D + 1                                         # 301 hm columns (ones col)

# engine assignment for the dst pre-step (per jj chunk): True -> gpsimd
DST_PRE_GP = [False, True, True, True]


def build_nc():
    nc = bacc.Bacc("TRN2", target_bir_lowering=False, debug=False,
                   enable_partition_id=True)

    def din(name, shape, dt=f32):
        return nc.dram_tensor(name, shape, dt, kind="ExternalInput").ap()

    identB = din("identB", [128, 128], bf16)
    w3cp = din("w3cp", [128, 3, DW], bf16)
    mcol = din("mcol", [128, GPC * 4])
    xT = din("xT", [GPC, 3, 128, E], bf16)        # input_state[n].T, padded
    adjAB = din("adjAB", [GPC, 2, E, E], fp8)     # [adj-2.5, adj.T-2.5]
    out = nc.dram_tensor("out", [GPC, 4, 128, D], bf16,
                         kind="ExternalOutput").ap()

    with tile.TileContext(nc) as tc:
        with ExitStack() as ctx:
            _body(ctx, tc, identB, w3cp, mcol, xT, adjAB, out)
    nc.compile()
    return nc


def _body(ctx, tc, identB, w3cp, mcol, xT, adjAB, out):
    nc = tc.nc
    HORNER = _register_horner()
    HORNERM = _register_hornerm()
    const = ctx.enter_context(tc.tile_pool(name="const", bufs=1))
    xpool = ctx.enter_context(tc.tile_pool(name="xpool", bufs=1))
    adjp = ctx.enter_context(tc.tile_pool(name="adjp", bufs=3))
    adjtp = ctx.enter_context(tc.tile_pool(name="adjtp", bufs=2))
    t1p = ctx.enter_context(tc.tile_pool(name="t1p", bufs=6))
    pip = ctx.enter_context(tc.tile_pool(name="pip", bufs=12))
    pbmp = ctx.enter_context(tc.tile_pool(name="pbmp", bufs=4))
    scp = ctx.enter_context(tc.tile_pool(name="scp", bufs=3))
    lrp = ctx.enter_context(tc.tile_pool(name="lrp", bufs=3))
    ehp = ctx.enter_context(tc.tile_pool(name="ehp", bufs=3))
    outp = ctx.enter_context(tc.tile_pool(name="outp", bufs=2))
    otp = ctx.enter_context(tc.tile_pool(name="otp", bufs=3))
    hmp = ctx.enter_context(tc.tile_pool(name="hmp", bufs=1))
    ckp = ctx.enter_context(tc.tile_pool(name="ckp", bufs=1))
    psq = ctx.enter_context(tc.tile_pool(name="psq", bufs=2, space="PSUM"))
    ztp = ctx.enter_context(tc.tile_pool(name="ztp", bufs=3, space="PSUM"))
    pop = ctx.enter_context(tc.tile_pool(name="pop", bufs=3, space="PSUM"))

    # ---- constant + weight loads (sync queue, in consumption order) ----
    IDB = const.tile([128, 128], bf16)
    nc.sync.dma_start(out=IDB, in_=identB)
    W3CP = const.tile([128, 3, DW], bf16)
    nc.sync.dma_start(out=W3CP, in_=w3cp)
    XTs = []
    for n in range(GPC):
        XTn = xpool.tile([128, 3, E], bf16, tag=f"xt_{n}")
        XTs.append(XTn)
    nc.sync.dma_start(out=XTs[0][:, :, 0:128],
                      in_=xT[0][:, :, 0:128].rearrange("c p e -> p c e"))
    nc.sync.dma_start(out=XTs[0][:, :, 128:E],
                      in_=xT[0][:, :, 128:E].rearrange("c p e -> p c e"))
    MC = const.tile([128, GPC * 4], f32)
    nc.sync.dma_start(out=MC, in_=mcol)
    # PE p-state warmup: dummy matmuls on the identity while inputs stream
    for w in range(10):
        pw = psq.tile([128, 128], f32, tag="ph", name="PW")
        nc.tensor.matmul(pw, IDB, IDB, start=True, stop=True,
                         skip_group_check=True)
    ABs = {}

    def load_adj(n):
        AB = adjp.tile([128, 2, 4, E], fp8, tag="ab", name="AB")
        nc.sync.dma_start(out=AB,
                          in_=adjAB[n].rearrange("b (c p) e -> p b c e", p=128))
        ABs[n] = AB

    def load_x(n):
        nc.sync.dma_start(out=XTs[n], in_=xT[n].rearrange("c p e -> p c e"))

    load_adj(0)
    load_x(1)
    load_adj(1)

    # hm tiles: ones column written once per buffer (16 tiles <-> 16 bufs)
    HMs, CKs = {}, {}
    for n in range(GPC):
        HMs[n] = []
        for ii in range(4):
            hm = hmp.tile([128, DH], bf16, tag=f"hm_{n}_{ii}")
            nc.gpsimd.memset(hm[:, D:DH], 1.0)
            HMs[n].append(hm)

    # ---- phase A: h3 + all poly coefficients in one matmul per e-chunk ----
    def emit_A(n):
        CK = []
        for ii, (eo, el) in enumerate(EC4):
            psh = psq.tile([128, DW], f32, tag="ph")
            for ci, (do, dl) in enumerate(DC3):
                nc.tensor.matmul(psh, XTs[n][:dl, ci, eo:eo + el],
                                 W3CP[:dl, ci, :],
                                 start=(ci == 0), stop=(ci == 2))
            hm = HMs[n][ii]
            nc.scalar.mul(hm[:, 0:D], psh[:, 0:D],
                          MC[:, n * 4 + ii:n * 4 + ii + 1])
            ck = ckp.tile([128, 8], f32, tag=f"ck_{n}_{ii}")
            nc.vector.tensor_copy(ck, psh[:, D + 8 * n:D + 8 * n + 8])
            CK.append(ck)
        CKs[n] = CK

    # ---- phase C: src cubic in [i-part, j-free] ----
    PSIs = {}

    def emit_C(n):
        AJ4 = ABs[n]
        PS_I = []
        for ii in range(4):
            aj = AJ4[:, 0, ii, :]
            ck = CKs[n][ii]
            t1 = t1p.tile([128, E], bf16, tag="t1")
            nc.gpsimd.tensor_scalar(t1, aj, ck[:, 3:4], ck[:, 2:3],
                                    Alu.mult, Alu.add)
            pi = pip.tile([128, E], bf16, tag="pi")
            nc.vector._custom_dve(HORNER, out=pi, in0=t1, in1=aj,
                                  s0=ck[:, 1:2], s1=ck[:, 0:1])
            PS_I.append(pi)
        PSIs[n] = PS_I

    # ---- phase T: PE transposes (bf16) into PSUM bf16 views ----
    ZTs = {}

    def emit_T(n):
        PS_I = PSIs[n]
        zv = []
        for half in range(2):
            ZT = ztp.tile([128, E], f32, tag="zt", name="ZT")
            ztv = ZT.bitcast(bf16)  # [128, 1024]
            for sub in range(2):
                jj = half * 2 + sub
                for ii, (eo, el) in enumerate(EC4):
                    nc.tensor.matmul(
                        ztv[:, sub * E + ii * 128: sub * E + ii * 128 + 128],
                        PS_I[ii][:, jj * 128:jj * 128 + 128], IDB,
                        is_transpose=True, start=True, stop=True,
                        skip_group_check=True)
            zv.append(ztv)
        ZTs[n] = zv

    # ---- phase D: dst cubic + combine + lrelu + exp in [j-part, i-free] ----
    EHs, SCs, LRs = {}, {}, {}

    def emit_D(n, half):
        AT4, CK = ABs[n], CKs[n]
        zv = ZTs[n]
        if half == 0:
            SCs[n] = scp.tile([128, 4, E], bf16, tag="sc", name="SC")
            LRs[n] = lrp.tile([128, 4, E], bf16, tag="lr", name="LR")
            EHs[n] = ehp.tile([128, 4, E], bf16, tag="eh", name="EH")
        SC, LR, EH = SCs[n], LRs[n], EHs[n]
        for jj in (2 * half, 2 * half + 1):
            at = AT4[:, 1, jj, :]
            ck = CK[jj]
            t1b = t1p.tile([128, E], bf16, tag="t1b")
            if DST_PRE_GP[jj] and n < 3:
                nc.gpsimd.tensor_scalar(t1b, at, ck[:, 7:8], ck[:, 6:7],
                                        Alu.mult, Alu.add)
            else:
                nc.vector.tensor_scalar(t1b, at, ck[:, 7:8], ck[:, 6:7],
                                        Alu.mult, Alu.add)
            pbm = pbmp.tile([128, E], bf16, tag="pbm")
            nc.vector._custom_dve(HORNERM, out=pbm, in0=t1b, in1=at,
                                  s0=ck[:, 5:6], s1=ck[:, 4:5], imm2=MASKVAL)
            ztv = zv[jj // 2]
            nc.vector.tensor_tensor(
                SC[:, jj, :],
                ztv[:, (jj % 2) * E:(jj % 2) * E + E].bitcast(bf16),
                pbm, Alu.add)
        h2 = 2 * half
        nc.scalar.activation(LR[:, h2:h2 + 2, :], SC[:, h2:h2 + 2, :],
                             Act.Prelu, alpha=SLOPE)
        nc.scalar.activation(EH[:, h2:h2 + 2, :], LR[:, h2:h2 + 2, :],
                             Act.Exp)

    # ---- phase E: final matmul + normalize (in ii-halves) ----
    OTs = {}

    def emit_E(n, half):
        EH, HM = EHs[n], HMs[n]
        if half == 0:
            OTs[n] = otp.tile([128, 4, D], bf16, tag="ot", name="OT")
        OT = OTs[n]
        i0, i1 = 2 * half, 2 * half + 1
        poA = pop.tile([128, DH], f32, tag="po", name="poA")
        poB = pop.tile([128, DH], f32, tag="po", name="poB")
        for jj in range(4):
            nc.tensor.matmul(poA, EH[:, jj, EC4[i0][0]:EC4[i0][0] + 128],
                             HM[jj][:, 0:DH],
                             start=(jj == 0), stop=(jj == 3),
                             skip_group_check=True)
            nc.tensor.matmul(poB, EH[:, jj, EC4[i1][0]:EC4[i1][0] + 128],
                             HM[jj][:, 0:DH],
                             start=(jj == 0), stop=(jj == 3),
                             skip_group_check=True)
        for ii, po in ((i0, poA), (i1, poB)):
            rc = outp.tile([128, 1], f32, tag="rc")
            nc.vector.reciprocal(rc, po[:, D:D + 1])
            nc.scalar.mul(OT[:, ii, :], po[:, 0:D], rc)
        nc.sync.dma_start(
            out=out[n, 2 * half:2 * half + 2].rearrange("c p d -> p c d"),
            in_=OT[:, 2 * half:2 * half + 2, :])

    # software-pipelined emission: 2 graphs in flight, half-graph granularity
    emit_A(0)
    emit_C(0)
    emit_A(1)
    emit_T(0)
    emit_D(0, 0)
    load_x(2)
    emit_C(1)
    emit_D(0, 1)
    load_adj(2)
    emit_T(1)
    emit_E(0, 0)
    emit_D(1, 0)
    emit_E(0, 1)
    emit_D(1, 1)
    emit_A(2)
    load_x(3)
    emit_C(2)
    load_adj(3)
    emit_T(2)
    emit_E(1, 0)
    emit_D(2, 0)
    emit_E(1, 1)
    emit_A(3)
    emit_D(2, 1)
    emit_C(3)
    emit_T(3)
    emit_D(3, 0)
    emit_E(2, 0)
    emit_D(3, 1)
    emit_E(2, 1)
    emit_E(3, 0)
    emit_E(3, 1)


def _vinv():
    # centered basis z = adj - 2.5: coeffs a0..a3 of the cubic through
    # (z_t, u_t), z_t in {-1.5,-0.5,0.5,1.5} (well conditioned, exact bf16)
    V = np.array([[((t + 1) - 2.5) ** m for m in range(4)] for t in range(4)],
                 np.float64)
    return np.linalg.inv(V)


def _prep_inputs(input_state, adj, node_mask, query_vec, W_type, a_type,
                 qattn_W1, qattn_W2):
    import ml_dtypes
    bf = ml_dtypes.bfloat16
    f8 = ml_dtypes.float8_e4m3fn
    X = np.asarray(input_state, np.float32)
    A = np.asarray(adj, np.int32)
    NMsk = np.asarray(node_mask, np.float32)
    Q = np.asarray(query_vec, np.float64)
    W = np.asarray(W_type, np.float64)
    AV = np.asarray(a_type, np.float64)
    W1 = np.asarray(qattn_W1, np.float64)
    W2 = np.asarray(qattn_W2, np.float64)

    # host: q-gate MLP + fold gate*a through W_t^T, then Vinv cubic basis
    Vsrc = np.zeros((N, NT, D))
    Vdst = np.zeros((N, NT, D))
    for t in range(NT):
        r = np.maximum(Q @ W1[t], 0.0)
        g = 1.0 / (1.0 + np.exp(-(r @ W2[t])))      # [N, 600]
        g1, g2 = g[:, :D], g[:, D:]
        a1, a2 = AV[t][:D], AV[t][D:]
        Vsrc[:, t] = (g1 * a1) @ W[t].T
        Vdst[:, t] = (g2 * a2) @ W[t].T
    Vi = _vinv()
    Csrc = np.einsum("kt,ntd->nkd", Vi, Vsrc)       # [N,4,300]
    Cdst = np.einsum("kt,ntd->nkd", Vi, Vdst)

    identB = np.ascontiguousarray(np.eye(128, dtype=np.float32)).astype(bf)
    ZA = (A.astype(np.float32) - 2.5).astype(f8)                  # [N,E,E]
    ZB = np.ascontiguousarray(
        (A.transpose(0, 2, 1).astype(np.float32) - 2.5)).astype(f8)
    ZAB = np.ascontiguousarray(np.stack([ZA, ZB], axis=1))        # [N,2,E,E]
    XP = np.zeros((N, 3, 128, E), np.float32)
    XT_ = X.transpose(0, 2, 1)                                    # [N,300,E]
    for ci, (do, dl) in enumerate(DC3):
        XP[:, ci, 0:dl, :] = XT_[:, do:do + dl, :]

    in_maps = []
    for c in range(NCORES):
        sl = slice(c * GPC, (c + 1) * GPC)
        # W3CP pack: W_3 rows + per-graph coefficient columns
        w3cp = np.zeros((128, 3, DW), np.float32)
        for ci, (do, dl) in enumerate(DC3):
            w3cp[:dl, ci, 0:D] = W[NT - 1, do:do + dl, :]
            for g in range(GPC):
                n = c * GPC + g
                for k in range(4):
                    w3cp[:dl, ci, D + 8 * g + k] = Csrc[n, k, do:do + dl]
                    w3cp[:dl, ci, D + 8 * g + 4 + k] = Cdst[n, k, do:do + dl]
        Mc = np.maximum(NMsk[sl, :, 0], 0.0)        # [GPC, 512]
        mcol = np.zeros((128, GPC * 4), np.float32)
        for g in range(GPC):
            for jj in range(4):
                mcol[:, g * 4 + jj] = Mc[g, jj * 128:(jj + 1) * 128]
        in_maps.append({
            "identB": identB,
            "w3cp": w3cp.astype(bf),
            "mcol": mcol,
            "xT": np.ascontiguousarray(XP[sl]).astype(bf),
            "adjAB": ZAB[sl],
        })
    return in_maps


_NC_CACHE = {}


def kernel(**inputs):
    if "nc" not in _NC_CACHE:
        _NC_CACHE["nc"] = build_nc()
    nc = _NC_CACHE["nc"]
    in_maps = _prep_inputs(**inputs)
    res = run_bass_kernel_spmd(nc, in_maps, list(range(NCORES)))
    outs = []
    for c in range(NCORES):
        o = np.asarray(res.results[c]["out"]).astype(np.float32)
        outs.append(o.reshape(GPC, E, D))
    return np.concatenate(outs, axis=0).astype(np.float32)


# revision 22
# speedup vs baseline: 1.0167x; 1.0154x over previous
"""GAT self-attention kernel for Trainium2 (8 NeuronCores, SPMD data-parallel over N).

Math (per graph n):
  h_t = X @ W_t ; q_gate_t = sigmoid(relu(q @ W1_t) @ W2_t)
  s_src_t = (h_t * g1) @ a1 ; s_dst_t = (h_t * g2) @ a2
  score[i,j] = lrelu(s_src_{adj[i,j]-1}[i] + s_dst_{adj[i,j]-1}[j])   (adj>0)
  out = softmax_j(score) @ (h_3 * node_mask)

Device strategy (v3):
  - The tiny q-gate MLP and the per-type fold (gate*a through W_t^T, then
    through the Vinv cubic-interpolation basis) run on the HOST; the device
    receives W3CP = [W_3 | per-graph src/dst cubic-coefficient columns] and
    computes h_3 plus all per-row polynomial coefficients in one fused matmul.
  - The 4-way type select over adj is a cubic in z = adj - 2.5 evaluated per
    cell: src side in [i-part, j-free] (ACT pre-step + custom DVE Horner,
    bf16), PE-transposed (bf16, cheap) into PSUM; dst side in [j-part, i-free]
    (gpsimd pre-step + custom DVE Horner with the adj>0 mask emitting -3e4).
  - combine is a builtin DVE tensor-tensor add (PSUM-bf16 + SBUF-bf16, 2x
    mode), then Prelu and Exp on the ACT engine (one table set, no reloads).
  - node_mask folds into hm = h_3 * m during the PSUM->SBUF copy (ACT Copy
    with per-partition scale); a ones column in hm recovers the softmax
    denominator through the final matmul; normalization happens on the host
    (the kernel DMAs numerator+denominator straight from PSUM as f32).
"""

import numpy as np
from contextlib import ExitStack

import concourse.bass as bass
import concourse.bacc as bacc
import concourse.tile as tile
from concourse import mybir
from concourse import dve_ops
from concourse.dve_spec import (Spec, Src0, Src1, C0, C1, C2, MaxNeg, Zero,
                                One, select)
from concourse.dve_uop import DveOpSpec
from concourse.bass_utils import run_bass_kernel_spmd


def _register_dve_op(name, spec):
    """Runtime-register a custom DVE op (fp32-internal fused pipeline)."""
    if name in dve_ops._SUB_OPCODE_FOR_NAME:
        return dve_ops.CUSTOM_DVE_SPECS[name + "_OP"]
    op = dve_ops.DveOp(name, spec, subdim=False, uops_sha={},
                       perf_en={"v3": True, "v4": True})
    dve_ops.OPS.append(op)
    dve_ops.CUSTOM_DVE_SPECS[name] = spec
    dve_ops._SUB_OPCODE_FOR_NAME[name] = (
        max(dve_ops._SUB_OPCODE_FOR_NAME.values()) + 1)
    shas = {}
    for ver in ("v3", "v4"):
        u = dve_ops.lower(spec, ver=ver)
        ds = DveOpSpec(
            name=name,
            opcode=dve_ops.get_dve_sub_opcode(name),
            uops=u,
            uops_2x=u,
            uops_2x_2p=u,
            perf_max=2,
            rd1_en=dve_ops.has_src1(spec),
        )
        dve_ops._COMPILE_CACHE[(name, ver)] = ds
        shas[ver] = ds.sha(ver)
    object.__setattr__(op, "uops_sha", shas)
    dve_ops.CUSTOM_DVE_SPECS[name + "_OP"] = op
    return op


def _register_horner():
    # out = (in0*in1 + s0)*in1 + s1 : cubic tail given t1 = a3*z + a2
    return _register_dve_op("HORNER2A_ANT", Spec(
        body=(Src0 * Src1 + C0) * Src1 + C1,
        reference=lambda in0, in1, s0, s1, imm2: (in0 * in1 + s0) * in1 + s1,
    ))


def _register_hornerm():
    # masked cubic tail: imm2 (a large negative, bf16-safe) where
    # in1 (= adj-2.5) <= -2, i.e. adj == 0
    return _register_dve_op("HORNERM_ANT", Spec(
        body=select(Src1 > (Zero - (One + One)),
                    (Src0 * Src1 + C0) * Src1 + C1, C2),
        reference=lambda in0, in1, s0, s1, imm2: np.where(
            in1 > -2.0, (in0 * in1 + s0) * in1 + s1, imm2),
    ))


f32 = mybir.dt.float32
f32r = mybir.dt.float32r
bf16 = mybir.dt.bfloat16
fp8 = mybir.dt.float8e4
Alu = mybir.AluOpType
Act = mybir.ActivationFunctionType

N, E, D, NT = 32, 512, 300, 4
NCORES = 8
GPC = N // NCORES  # graphs per core
SLOPE = 0.2
MASKVAL = -30000.0

DC3 = [(0, 128), (128, 128), (256, 44)]           # 300 split into <=128 chunks
EC4 = [(i * 128, 128) for i in range(4)]          # 512 split into 4 chunks
DW = D + 32                                        # 332 W3CP columns
DH = D + 1                                         # 301 hm columns (ones col)

# engine assignment for the dst pre-step (per jj chunk): True -> gpsimd
DST_PRE_GP = [False, True, True, True]


def build_nc():
    nc = bacc.Bacc("TRN2", target_bir_lowering=False, debug=False,
                   enable_partition_id=True)

    def din(name, shape, dt=f32):
        return nc.dram_tensor(name, shape, dt, kind="ExternalInput").ap()

    identB = din("identB", [128, 128], bf16)
    w3cp = din("w3cp", [128, 3, DW], bf16)
    mcol = din("mcol", [128, GPC * 4])
    xT = din("xT", [GPC, 3, 128, E], bf16)        # input_state[n].T, padded
    adjAB = din("adjAB", [GPC, 2, E, E], fp8)     # [adj-2.5, adj.T-2.5]
    out = nc.dram_tensor("out", [GPC, 4, 128, D], bf16,
                         kind="ExternalOutput").ap()

    with tile.TileContext(nc) as tc:
        with ExitStack() as ctx:
            _body(ctx, tc, identB, w3cp, mcol, xT, adjAB, out)
    nc.compile()
    return nc


def _body(ctx, tc, identB, w3cp, mcol, xT, adjAB, out):
    nc = tc.nc
    HORNER = _register_horner()
    HORNERM = _register_hornerm()
    const = ctx.enter_context(tc.tile_pool(name="const", bufs=1))
    xpool = ctx.enter_context(tc.tile_pool(name="xpool", bufs=1))
    adjp = ctx.enter_context(tc.tile_pool(name="adjp", bufs=3))
    adjtp = ctx.enter_context(tc.tile_pool(name="adjtp", bufs=2))
    t1p = ctx.enter_context(tc.tile_pool(name="t1p", bufs=6))
    pip = ctx.enter_context(tc.tile_pool(name="pip", bufs=12))
    pbmp = ctx.enter_context(tc.tile_pool(name="pbmp", bufs=4))
    scp = ctx.enter_context(tc.tile_pool(name="scp", bufs=3))
    lrp = ctx.enter_context(tc.tile_pool(name="lrp", bufs=3))
    ehp = ctx.enter_context(tc.tile_pool(name="ehp", bufs=3))
    outp = ctx.enter_context(tc.tile_pool(name="outp", bufs=2))
    otp = ctx.enter_context(tc.tile_pool(name="otp", bufs=3))
    hmp = ctx.enter_context(tc.tile_pool(name="hmp", bufs=1))
    ckp = ctx.enter_context(tc.tile_pool(name="ckp", bufs=1))
    psq = ctx.enter_context(tc.tile_pool(name="psq", bufs=2, space="PSUM"))
    ztp = ctx.enter_context(tc.tile_pool(name="ztp", bufs=3, space="PSUM"))
    pop = ctx.enter_context(tc.tile_pool(name="pop", bufs=3, space="PSUM"))

    # ---- constant + weight loads (sync queue, in consumption order) ----
    IDB = const.tile([128, 128], bf16)
    nc.sync.dma_start(out=IDB, in_=identB)
    W3CP = const.tile([128, 3, DW], bf16)
    nc.sync.dma_start(out=W3CP, in_=w3cp)
    XTs = []
    for n in range(GPC):
        XTn = xpool.tile([128, 3, E], bf16, tag=f"xt_{n}")
        XTs.append(XTn)
    nc.sync.dma_start(out=XTs[0][:, :, 0:128],
                      in_=xT[0][:, :, 0:128].rearrange("c p e -> p c e"))
    nc.sync.dma_start(out=XTs[0][:, :, 128:E],
                      in_=xT[0][:, :, 128:E].rearrange("c p e -> p c e"))
    MC = const.tile([128, GPC * 4], f32)
    nc.sync.dma_start(out=MC, in_=mcol)
    # PE p-state warmup: dummy matmuls on the identity while inputs stream
    for w in range(10):
        pw = psq.tile([128, 128], f32, tag="ph", name="PW")
        nc.tensor.matmul(pw, IDB, IDB, start=True, stop=True,
                         skip_group_check=True)
    ABs = {}

    def load_adj(n):
        AB = adjp.tile([128, 2, 4, E], fp8, tag="ab", name="AB")
        nc.sync.dma_start(out=AB,
                          in_=adjAB[n].rearrange("b (c p) e -> p b c e", p=128))
        ABs[n] = AB

    def load_x(n):
        nc.sync.dma_start(out=XTs[n], in_=xT[n].rearrange("c p e -> p c e"))

    load_adj(0)
    load_x(1)
    load_adj(1)

    # hm tiles: ones column written once per buffer (16 tiles <-> 16 bufs)
    HMs, CKs = {}, {}
    for n in range(GPC):
        HMs[n] = []
        for ii in range(4):
            hm = hmp.tile([128, DH], bf16, tag=f"hm_{n}_{ii}")
            nc.gpsimd.memset(hm[:, D:DH], 1.0)
            HMs[n].append(hm)

    # ---- phase A: h3 + all poly coefficients in one matmul per e-chunk ----
    def emit_A(n):
        CK = []
        for ii, (eo, el) in enumerate(EC4):
            psh = psq.tile([128, DW], f32, tag="ph")
            for ci, (do, dl) in enumerate(DC3):
                nc.tensor.matmul(psh, XTs[n][:dl, ci, eo:eo + el],
                                 W3CP[:dl, ci, :],
                                 start=(ci == 0), stop=(ci == 2))
            hm = HMs[n][ii]
            nc.scalar.mul(hm[:, 0:D], psh[:, 0:D],
                          MC[:, n * 4 + ii:n * 4 + ii + 1])
            ck = ckp.tile([128, 8], f32, tag=f"ck_{n}_{ii}")
            nc.vector.tensor_copy(ck, psh[:, D + 8 * n:D + 8 * n + 8])
            CK.append(ck)
        CKs[n] = CK

    # ---- phase C: src cubic in [i-part, j-free] ----
    PSIs = {}

    def emit_C(n):
        AJ4 = ABs[n]
        PS_I = []
        for ii in range(4):
            aj = AJ4[:, 0, ii, :]
            ck = CKs[n][ii]
            t1 = t1p.tile([128, E], bf16, tag="t1")
            nc.gpsimd.tensor_scalar(t1, aj, ck[:, 3:4], ck[:, 2:3],
                                    Alu.mult, Alu.add)
            pi = pip.tile([128, E], bf16, tag="pi")
            nc.vector._custom_dve(HORNER, out=pi, in0=t1, in1=aj,
                                  s0=ck[:, 1:2], s1=ck[:, 0:1])
            PS_I.append(pi)
        PSIs[n] = PS_I

    # ---- phase T: PE transposes (bf16) into PSUM bf16 views ----
    ZTs = {}

    def emit_T(n):
        PS_I = PSIs[n]
        zv = []
        for half in range(2):
            ZT = ztp.tile([128, E], f32, tag="zt", name="ZT")
            ztv = ZT.bitcast(bf16)  # [128, 1024]
            for sub in range(2):
                jj = half * 2 + sub
                for ii, (eo, el) in enumerate(EC4):
                    nc.tensor.matmul(
                        ztv[:, sub * E + ii * 128: sub * E + ii * 128 + 128],
                        PS_I[ii][:, jj * 128:jj * 128 + 128], IDB,
                        is_transpose=True, start=True, stop=True,
                        skip_group_check=True)
            zv.append(ztv)
        ZTs[n] = zv

    # ---- phase D: dst cubic + combine + lrelu + exp in [j-part, i-free] ----
    EHs, SCs, LRs = {}, {}, {}

    def emit_D(n, half):
        AT4, CK = ABs[n], CKs[n]
        zv = ZTs[n]
        if half == 0:
            SCs[n] = scp.tile([128, 4, E], bf16, tag="sc", name="SC")
            LRs[n] = lrp.tile([128, 4, E], bf16, tag="lr", name="LR")
            EHs[n] = ehp.tile([128, 4, E], bf16, tag="eh", name="EH")
        SC, LR, EH = SCs[n], LRs[n], EHs[n]
        for jj in (2 * half, 2 * half + 1):
            at = AT4[:, 1, jj, :]
            ck = CK[jj]
            t1b = t1p.tile([128, E], bf16, tag="t1b")
            if DST_PRE_GP[jj] and n < 3:
                nc.gpsimd.tensor_scalar(t1b, at, ck[:, 7:8], ck[:, 6:7],
                                        Alu.mult, Alu.add)
            else:
                nc.vector.tensor_scalar(t1b, at, ck[:, 7:8], ck[:, 6:7],
                                        Alu.mult, Alu.add)
            pbm = pbmp.tile([128, E], bf16, tag="pbm")
            nc.vector._custom_dve(HORNERM, out=pbm, in0=t1b, in1=at,
                                  s0=ck[:, 5:6], s1=ck[:, 4:5], imm2=MASKVAL)
            ztv = zv[jj // 2]
            nc.vector.tensor_tensor(
                SC[:, jj, :],
                ztv[:, (jj % 2) * E:(jj % 2) * E + E].bitcast(bf16),
                pbm, Alu.add)
        h2 = 2 * half
        nc.scalar.activation(LR[:, h2:h2 + 2, :], SC[:, h2:h2 + 2, :],
                             Act.Prelu, alpha=SLOPE)
        nc.scalar.activation(EH[:, h2:h2 + 2, :], LR[:, h2:h2 + 2, :],
                             Act.Exp)

    # ---- phase E: final matmul + normalize (in ii-halves) ----
    OTs = {}

    def emit_E(n, half):
        EH, HM = EHs[n], HMs[n]
        if half == 0:
            OTs[n] = otp.tile([128, 4, D], bf16, tag="ot", name="OT")
        OT = OTs[n]
        i0, i1 = 2 * half, 2 * half + 1
        poA = pop.tile([128, DH], f32, tag="po", name="poA")
        poB = pop.tile([128, DH], f32, tag="po", name="poB")
        for jj in range(4):
            nc.tensor.matmul(poA, EH[:, jj, EC4[i0][0]:EC4[i0][0] + 128],
                             HM[jj][:, 0:DH],
                             start=(jj == 0), stop=(jj == 3),
                             skip_group_check=True)
            nc.tensor.matmul(poB, EH[:, jj, EC4[i1][0]:EC4[i1][0] + 128],
                             HM[jj][:, 0:DH],
                             start=(jj == 0), stop=(jj == 3),
                             skip_group_check=True)
        for ii, po in ((i0, poA), (i1, poB)):
            rc = outp.tile([128, 1], f32, tag="rc")
            nc.vector.reciprocal(rc, po[:, D:D + 1])
            nc.scalar.mul(OT[:, ii, :], po[:, 0:D], rc)
        nc.sync.dma_start(
            out=out[n, 2 * half:2 * half + 2].rearrange("c p d -> p c d"),
            in_=OT[:, 2 * half:2 * half + 2, :])

    # software-pipelined emission: 2 graphs in flight, half-graph granularity
    emit_A(0)
    emit_C(0)
    emit_A(1)
    emit_T(0)
    emit_D(0, 0)
    load_x(2)
    emit_C(1)
    emit_D(0, 1)
    load_adj(2)
    emit_T(1)
    emit_E(0, 0)
    emit_D(1, 0)
    emit_E(0, 1)
    emit_D(1, 1)
    emit_A(2)
    load_x(3)
    emit_C(2)
    load_adj(3)
    emit_T(2)
    emit_E(1, 0)
    emit_D(2, 0)
    emit_E(1, 1)
    emit_A(3)
    emit_D(2, 1)
    emit_C(3)
    emit_T(3)
    emit_D(3, 0)
    emit_E(2, 0)
    emit_D(3, 1)
    emit_E(2, 1)
    emit_E(3, 0)
    emit_E(3, 1)


def _vinv():
    # centered basis z = adj - 2.5: coeffs a0..a3 of the cubic through
    # (z_t, u_t), z_t in {-1.5,-0.5,0.5,1.5} (well conditioned, exact bf16)
    V = np.array([[((t + 1) - 2.5) ** m for m in range(4)] for t in range(4)],
                 np.float64)
    return np.linalg.inv(V)


def _prep_inputs(input_state, adj, node_mask, query_vec, W_type, a_type,
                 qattn_W1, qattn_W2):
    import ml_dtypes
    bf = ml_dtypes.bfloat16
    f8 = ml_dtypes.float8_e4m3fn
    X = np.asarray(input_state, np.float32)
    A = np.asarray(adj, np.int32)
    NMsk = np.asarray(node_mask, np.float32)
    Q = np.asarray(query_vec, np.float64)
    W = np.asarray(W_type, np.float64)
    AV = np.asarray(a_type, np.float64)
    W1 = np.asarray(qattn_W1, np.float64)
    W2 = np.asarray(qattn_W2, np.float64)

    # host: q-gate MLP + fold gate*a through W_t^T, then Vinv cubic basis
    Vsrc = np.zeros((N, NT, D))
    Vdst = np.zeros((N, NT, D))
    for t in range(NT):
        r = np.maximum(Q @ W1[t], 0.0)
        g = 1.0 / (1.0 + np.exp(-(r @ W2[t])))      # [N, 600]
        g1, g2 = g[:, :D], g[:, D:]
        a1, a2 = AV[t][:D], AV[t][D:]
        Vsrc[:, t] = (g1 * a1) @ W[t].T
        Vdst[:, t] = (g2 * a2) @ W[t].T
    Vi = _vinv()
    Csrc = np.einsum("kt,ntd->nkd", Vi, Vsrc)       # [N,4,300]
    Cdst = np.einsum("kt,ntd->nkd", Vi, Vdst)

    identB = np.ascontiguousarray(np.eye(128, dtype=np.float32)).astype(bf)
    ZA = (A.astype(np.float32) - 2.5).astype(f8)                  # [N,E,E]
    ZB = np.ascontiguousarray(
        (A.transpose(0, 2, 1).astype(np.float32) - 2.5)).astype(f8)
    ZAB = np.ascontiguousarray(np.stack([ZA, ZB], axis=1))        # [N,2,E,E]
    XP = np.zeros((N, 3, 128, E), np.float32)
    XT_ = X.transpose(0, 2, 1)                                    # [N,300,E]
    for ci, (do, dl) in enumerate(DC3):
        XP[:, ci, 0:dl, :] = XT_[:, do:do + dl, :]

    in_maps = []
    for c in range(NCORES):
        sl = slice(c * GPC, (c + 1) * GPC)
        # W3CP pack: W_3 rows + per-graph coefficient columns
        w3cp = np.zeros((128, 3, DW), np.float32)
        for ci, (do, dl) in enumerate(DC3):
            w3cp[:dl, ci, 0:D] = W[NT - 1, do:do + dl, :]
            for g in range(GPC):
                n = c * GPC + g
                for k in range(4):
                    w3cp[:dl, ci, D + 8 * g + k] = Csrc[n, k, do:do + dl]
                    w3cp[:dl, ci, D + 8 * g + 4 + k] = Cdst[n, k, do:do + dl]
        Mc = np.maximum(NMsk[sl, :, 0], 0.0)        # [GPC, 512]
        mcol = np.zeros((128, GPC * 4), np.float32)
        for g in range(GPC):
            for jj in range(4):
                mcol[:, g * 4 + jj] = Mc[g, jj * 128:(jj + 1) * 128]
        in_maps.append({
            "identB": identB,
            "w3cp": w3cp.astype(bf),
            "mcol": mcol,
            "xT": np.ascontiguousarray(XP[sl]).astype(bf),
            "adjAB": ZAB[sl],
        })
    return in_maps


_NC_CACHE = {}


def kernel(**inputs):
    if "nc" not in _NC_CACHE:
        _NC_CACHE["nc"] = build_nc()
    nc = _NC_CACHE["nc"]
    in_maps = _prep_inputs(**inputs)
    res = run_bass_kernel_spmd(nc, in_maps, list(range(NCORES)))
    outs = []
    for c in range(NCORES):
        o = np.asarray(res.results[c]["out"]).astype(np.float32)
        outs.append(o.reshape(GPC, E, D))
    return np.concatenate(outs, axis=0).astype(np.float32)
